# revision 9
# baseline (speedup 1.0000x reference)
"""Multi-head self-attention Trainium2 kernel (8 NeuronCores, head-parallel).

B=2, S=2048, D=1024, H=16, DK=64. Each core owns 2 heads (a 128-wide slice of
the QKV output dims / Wo input dims) and both batch elements. Inputs are
pre-sliced + pre-transposed on the host; partial Wo outputs are summed on the
host. No cross-core communication.

Device-side layout (all bf16 on PE, fp32 PSUM accumulation):
  xT   [D, T]        activations feature-major (T = B*S tokens)
  QT,KT[128, T]      per-core head dims on partitions
  V    [tok, dk]     token-major, augmented with a ones column per head so the
                     attn @ V matmul also yields softmax denominators
  scores/exp/attn    [k, q] tiles (k on partitions) -> feeds attn @ V directly
  ctxT [128, T]      context feature-major -> feeds Wo projection
  outT [D, T]        per-core partial of the final projection
"""

import os
import sys

import numpy as np

for _p in ("/opt/trn_rl_repo",):
    if _p not in sys.path and os.path.isdir(_p):
        sys.path.insert(0, _p)

import ml_dtypes  # noqa: E402

import concourse.bass as bass  # noqa: E402
import concourse.mybir as mybir  # noqa: E402
import concourse.tile as tile  # noqa: E402
from concourse import bacc  # noqa: E402

BF16 = ml_dtypes.bfloat16
P = 128

# Problem config (hardcoded per spec)
B, S, D, H = 2, 2048, 1024, 16
DK = D // H            # 64
N_CORES = 8
HPC = H // N_CORES     # heads per core = 2
DSH = HPC * DK         # head dims per core = 128
T = B * S              # 4096 tokens
VW = DSH + HPC         # V width incl. ones cols = 130

TRACE = False
LAST_RESULTS = None


def build_program(q_chunk=1024):
    """Build the single-core Bass program (same program on all 8 cores)."""
    f32 = mybir.dt.float32
    bf16 = mybir.dt.bfloat16
    DT = D // P            # 8 d-tiles (contraction over D)
    KT = S // P            # 16 k-token tiles per batch
    TTI = T // P           # 32 token tiles total
    NOT = D // P           # 8 output tiles for Wo projection
    QC = S // q_chunk      # q chunks per batch
    QH = q_chunk // 512    # 512-wide halves per q chunk

    nc = bacc.Bacc(None, target_bir_lowering=False, debug=False)

    # ---- I/O ----
    xt_d = nc.declare_dram_parameter("xt", [D, T], bf16, isOutput=False)
    wq_d = nc.declare_dram_parameter("wq", [D, DSH], bf16, isOutput=False)
    bq_d = nc.declare_dram_parameter("bq", [DSH, 1], f32, isOutput=False)
    wk_d = nc.declare_dram_parameter("wk", [D, DSH], bf16, isOutput=False)
    bk_d = nc.declare_dram_parameter("bk", [DSH, 1], f32, isOutput=False)
    wv_d = nc.declare_dram_parameter("wv", [D, VW], bf16, isOutput=False)
    bv_d = nc.declare_dram_parameter("bv", [1, VW], bf16, isOutput=False)
    wo_d = nc.declare_dram_parameter("wo", [DSH, D], bf16, isOutput=False)
    bo_d = nc.declare_dram_parameter("bo", [P, NOT], f32, isOutput=False)
    attn_d = nc.declare_dram_parameter("attn", [B, HPC, S, S], bf16, isOutput=True)
    out_d = nc.declare_dram_parameter("out", [D, T], bf16, isOutput=True)

    AF = mybir.ActivationFunctionType
    OP = mybir.AluOpType

    with tile.TileContext(nc) as tc:
        with (
            tc.tile_pool(name="const", bufs=1) as cpool,
            tc.tile_pool(name="big", bufs=1) as bpool,
        ):
            # persistent SBUF tensors
            wq_sb = cpool.tile([P, DT, DSH], bf16, tag="wq")
            wk_sb = cpool.tile([P, DT, DSH], bf16, tag="wk")
            wv_sb = cpool.tile([P, DT, VW], bf16, tag="wv")
            wo_sb = cpool.tile([P, D], bf16, tag="wo")
            bq_sb = cpool.tile([P, 1], f32, tag="bq")
            bk_sb = cpool.tile([P, 1], f32, tag="bk")
            bo_sb = cpool.tile([P, NOT], f32, tag="bo")
            bvb_sb = cpool.tile([P, VW], bf16, tag="bvb")

            qt_sb = bpool.tile([P, T], bf16, tag="qt")
            kt_sb = bpool.tile([P, T], bf16, tag="kt")
            v_sb = bpool.tile([P, TTI, VW], bf16, tag="v")
            ctxt_sb = bpool.tile([P, T], bf16, tag="ctxt")

            nc.sync.dma_start(wq_sb[:], wq_d.rearrange("(dt p) m -> p dt m", p=P))
            nc.sync.dma_start(wk_sb[:], wk_d.rearrange("(dt p) m -> p dt m", p=P))
            nc.sync.dma_start(wv_sb[:], wv_d.rearrange("(dt p) m -> p dt m", p=P))
            nc.sync.dma_start(wo_sb[:], wo_d[:])
            nc.sync.dma_start(bq_sb[:], bq_d[:])
            nc.sync.dma_start(bk_sb[:], bk_d[:])
            nc.sync.dma_start(bo_sb[:], bo_d[:])
            nc.sync.dma_start(bvb_sb[:], bv_d[:].to_broadcast((P, VW)))

            # ACT warmup: force the exp table-set load onto a wait-free inst
            warm_sb = cpool.tile([1, 8], f32, tag="warm")
            nc.vector.memset(warm_sb[:], 0.0)
            nc.scalar.activation(warm_sb[:], warm_sb[:], AF.Exp)

            # ---------------- Phase A: projections ----------------
            with (
                tc.tile_pool(name="xt", bufs=1) as xpool,
                tc.tile_pool(name="psA", bufs=2, space="PSUM") as psA,
            ):
                xt_sb = xpool.tile([P, DT, T], bf16, tag="xt")
                nc.sync.dma_start(xt_sb[:], xt_d.rearrange("(dt p) t -> p dt t", p=P))

                # QT, KT: [dsh, T], accumulate over 8 d-tiles per 512-chunk
                for w_sb, b_sb, dst in ((wq_sb, bq_sb, qt_sb), (wk_sb, bk_sb, kt_sb)):
                    for ch in range(T // 512):
                        ps = psA.tile([P, 512], f32, tag="psqk")
                        sl = slice(ch * 512, (ch + 1) * 512)
                        for dt in range(DT):
                            nc.tensor.matmul(
                                ps[:],
                                lhsT=w_sb[:, dt, :],
                                rhs=xt_sb[:, dt, sl],
                                start=(dt == 0),
                                stop=(dt == DT - 1),
                            )
                        nc.vector.tensor_tensor(
                            dst[:, sl],
                            ps[:],
                            b_sb[:, 0:1].to_broadcast((P, 512)),
                            OP.add,
                        )

                # V: token-major [tok, VW] with ones cols; bias via broadcast add
                for tt in range(TTI):
                    psv = psA.tile([P, VW], f32, tag="psv")
                    tsl = slice(tt * P, (tt + 1) * P)
                    for dt in range(DT):
                        nc.tensor.matmul(
                            psv[:],
                            lhsT=xt_sb[:, dt, tsl],
                            rhs=wv_sb[:, dt, :],
                            start=(dt == 0),
                            stop=(dt == DT - 1),
                        )
                    nc.vector.tensor_tensor(v_sb[:, tt, :], psv[:], bvb_sb[:], OP.add)

            # ---------------- Phase B: attention ----------------
            with (
                tc.tile_pool(name="exp", bufs=2) as epool,
                tc.tile_pool(name="bc", bufs=1) as bcpool,
                tc.tile_pool(name="dbc", bufs=2, space="DRAM") as dpool,
                tc.tile_pool(name="psS", bufs=1, space="PSUM") as psS,
                tc.tile_pool(name="psC", bufs=1, space="PSUM") as psC,
            ):
                for b in range(B):
                    for qc in range(QC):
                        q0 = b * S + qc * q_chunk  # token offset of q chunk
                        exps = [
                            epool.tile([P, KT, q_chunk], bf16, tag=f"exp{hl}", name=f"exp{hl}")
                            for hl in range(HPC)
                        ]
                        # scores + exp, k-tile by k-tile, both heads row-packed
                        for kt in range(KT):
                            ksl = slice(b * S + kt * P, b * S + (kt + 1) * P)
                            pss = [
                                psS.tile([P, q_chunk], f32, tag=f"S{hl}", name=f"S{hl}")
                                for hl in range(HPC)
                            ]
                            for hl in range(HPC):
                                hp = slice(hl * DK, (hl + 1) * DK)
                                for qh in range(QH):
                                    qsl = slice(q0 + qh * 512, q0 + (qh + 1) * 512)
                                    nc.tensor.matmul(
                                        pss[hl][:, qh * 512 : (qh + 1) * 512],
                                        lhsT=kt_sb[hp, ksl],
                                        rhs=qt_sb[hp, qsl],
                                        start=True,
                                        stop=True,
                                        tile_position=(hl * DK, 0),
                                    )
                            for hl in range(HPC):
                                nc.scalar.activation(
                                    exps[hl][:, kt, :], pss[hl][:], AF.Exp
                                )
                        # attn @ [V | 1]: accumulate ctx + row sums over k-tiles
                        cacc = [
                            [
                                psC.tile([DK + 1, 512], f32, tag=f"C{hl}{qh}", name=f"C{hl}{qh}")
                                for qh in range(QH)
                            ]
                            for hl in range(HPC)
                        ]
                        for kt in range(KT):
                            vt = b * KT + kt
                            for hl in range(HPC):
                                vsl = slice(hl * (DK + 1), (hl + 1) * (DK + 1))
                                for qh in range(QH):
                                    nc.tensor.matmul(
                                        cacc[hl][qh][:],
                                        lhsT=v_sb[:, vt, vsl],
                                        rhs=exps[hl][:, kt, qh * 512 : (qh + 1) * 512],
                                        start=(kt == 0),
                                        stop=(kt == KT - 1),
                                    )
                        # normalize + writeout per head
                        for hl in range(HPC):
                            sums = bcpool.tile([1, q_chunk], f32, tag=f"sum{hl}")
                            bcf = bcpool.tile([P, q_chunk], f32, tag=f"bcf{hl}")
                            bc16 = bcpool.tile([P, q_chunk], bf16, tag=f"bc16{hl}")
                            for qh in range(QH):
                                nc.vector.tensor_copy(
                                    out=sums[0:1, qh * 512 : (qh + 1) * 512],
                                    in_=cacc[hl][qh][DK : DK + 1, :],
                                )
                            sums_dr = dpool.tile(
                                [1, q_chunk], f32, tag=f"sd{hl}", name=f"sd{hl}"
                            )
                            nc.sync.dma_start(sums_dr[:], sums[0:1, :])
                            nc.sync.dma_start(
                                bcf[:], sums_dr[:].to_broadcast((P, q_chunk))
                            )
                            nc.vector.reciprocal_approx_fast(out=bcf[:], in_=bcf[:])
                            nc.vector.tensor_copy(out=bc16[:], in_=bcf[:])
                            # attn = exp * (1/sum), in place, then DMA out
                            nc.vector.tensor_tensor(
                                exps[hl][:],
                                exps[hl][:],
                                bc16[:, None, :].to_broadcast((P, KT, q_chunk)),
                                OP.mult,
                            )
                            nc.sync.dma_start(
                                attn_d[b, hl, :, qc * q_chunk : (qc + 1) * q_chunk]
                                .rearrange("(kt p) q -> p kt q", p=P),
                                exps[hl][:],
                            )
                            # ctxT = unnorm_ctx * (1/sum)
                            for qh in range(QH):
                                csl = slice(q0 + qh * 512, q0 + (qh + 1) * 512)
                                nc.vector.tensor_tensor(
                                    ctxt_sb[hl * DK : (hl + 1) * DK, csl],
                                    cacc[hl][qh][0:DK, :],
                                    bc16[0:DK, qh * 512 : (qh + 1) * 512],
                                    OP.mult,
                                )

            # ---------------- Phase C: output projection ----------------
            with (
                tc.tile_pool(name="psO", bufs=2, space="PSUM") as psO,
                tc.tile_pool(name="ost", bufs=3) as opool,
            ):
                for ot in range(NOT):
                    osl = slice(ot * P, (ot + 1) * P)
                    for ch in range(T // 512):
                        sl = slice(ch * 512, (ch + 1) * 512)
                        pso = psO.tile([P, 512], f32, tag="pso")
                        nc.tensor.matmul(
                            pso[:],
                            lhsT=wo_sb[:, osl],
                            rhs=ctxt_sb[:, sl],
                            start=True,
                            stop=True,
                        )
                        ou = opool.tile([P, 512], bf16, tag="ou")
                        nc.vector.tensor_tensor(
                            ou[:],
                            pso[:],
                            bo_sb[:, ot : ot + 1].to_broadcast((P, 512)),
                            OP.add,
                        )
                        nc.sync.dma_start(out_d[osl, sl], ou[:])

    nc.compile()
    return nc


_NC_CACHE = {}


def _get_program():
    if "nc" not in _NC_CACHE:
        _NC_CACHE["nc"] = build_program()
    return _NC_CACHE["nc"]


def _prep_inputs(x, Wq, bq, Wk, bk, Wv, bv, Wo, bo):
    """Host-side sharding + layout prep. Returns in_maps (one dict per core)."""
    scale = 1.0 / np.sqrt(np.float32(DK))  # folded into Wq/bq
    xt = np.ascontiguousarray(x.reshape(T, D).T).astype(BF16)
    in_maps = []
    for c in range(N_CORES):
        sl = slice(DSH * c, DSH * (c + 1))
        wq_c = np.ascontiguousarray((Wq[sl] * scale).T).astype(BF16)
        bq_c = (bq[sl] * scale).astype(np.float32)[:, None]
        wk_c = np.ascontiguousarray(Wk[sl].T).astype(BF16)
        bk_c = bk[sl].astype(np.float32)[:, None]
        wv_c = np.zeros((D, VW), np.float32)
        bv_c = np.zeros((1, VW), np.float32)
        for hl in range(HPC):
            rows = slice(DSH * c + hl * DK, DSH * c + (hl + 1) * DK)
            cols = slice(hl * (DK + 1), hl * (DK + 1) + DK)
            wv_c[:, cols] = Wv[rows].T
            bv_c[0, cols] = bv[rows]
            bv_c[0, hl * (DK + 1) + DK] = 1.0
        wo_c = np.ascontiguousarray(Wo[:, sl].T).astype(BF16)
        bo_c = np.ascontiguousarray((bo / N_CORES).reshape(D // P, P).T).astype(
            np.float32
        )
        in_maps.append(
            {
                "xt": xt,
                "wq": wq_c,
                "bq": bq_c,
                "wk": wk_c,
                "bk": bk_c,
                "wv": wv_c.astype(BF16),
                "bv": bv_c.astype(BF16),
                "wo": wo_c,
                "bo": bo_c,
            }
        )
    return in_maps


def kernel(x, Wq, bq, Wk, bk, Wv, bv, Wo, bo):
    global LAST_RESULTS
    from concourse.bass_utils import run_bass_kernel_spmd

    x = np.asarray(x, np.float32)
    in_maps = _prep_inputs(
        x,
        np.asarray(Wq, np.float32),
        np.asarray(bq, np.float32),
        np.asarray(Wk, np.float32),
        np.asarray(bk, np.float32),
        np.asarray(Wv, np.float32),
        np.asarray(bv, np.float32),
        np.asarray(Wo, np.float32),
        np.asarray(bo, np.float32),
    )
    nc = _get_program()
    res = run_bass_kernel_spmd(nc, in_maps, list(range(N_CORES)), trace=TRACE)
    LAST_RESULTS = res

    # ---- gather / unshard ----
    out_t = np.zeros((D, T), np.float32)
    attn = np.empty((B, H, S, S), np.float32)
    for c in range(N_CORES):
        r = res.results[c]
        out_t += np.asarray(r["out"]).astype(np.float32)
        a = np.asarray(r["attn"])  # [B, HPC, S(k), S(q)] bf16
        for hl in range(HPC):
            attn[:, HPC * c + hl] = np.swapaxes(
                a[:, hl].astype(np.float32), 1, 2
            )
    output = np.ascontiguousarray(out_t.T).reshape(B, S, D)
    return output, attn


# revision 14
# speedup vs baseline: 1.0068x; 1.0068x over previous
"""Multi-head self-attention Trainium2 kernel (8 NeuronCores, head-parallel).

B=2, S=2048, D=1024, H=16, DK=64. Each core owns 2 heads (a 128-wide slice of
the QKV output dims / Wo input dims) and both batch elements. Inputs are
pre-sliced + pre-transposed on the host; partial Wo outputs are summed on the
host. No cross-core communication.

Device-side layout (all bf16 on PE, fp32 PSUM accumulation):
  xT   [D, T]        activations feature-major (T = B*S tokens)
  QT,KT[128, T]      per-core head dims on partitions
  V    [tok, dk]     token-major, augmented with a ones column per head so the
                     attn @ V matmul also yields softmax denominators
  scores/exp/attn    [k, q] tiles (k on partitions) -> feeds attn @ V directly
  ctxT [128, T]      context feature-major -> feeds Wo projection
  outT [D, T]        per-core partial of the final projection
"""

import os
import sys

import numpy as np

for _p in ("/opt/trn_rl_repo",):
    if _p not in sys.path and os.path.isdir(_p):
        sys.path.insert(0, _p)

import ml_dtypes  # noqa: E402

import concourse.bass as bass  # noqa: E402
import concourse.mybir as mybir  # noqa: E402
import concourse.tile as tile  # noqa: E402
from concourse import bacc  # noqa: E402

BF16 = ml_dtypes.bfloat16
P = 128

# Problem config (hardcoded per spec)
B, S, D, H = 2, 2048, 1024, 16
DK = D // H            # 64
N_CORES = 8
HPC = H // N_CORES     # heads per core = 2
DSH = HPC * DK         # head dims per core = 128
T = B * S              # 4096 tokens
VW = DSH + HPC         # V width incl. ones cols = 130

TRACE = False
LAST_RESULTS = None


def build_program(q_chunk=1024):
    """Build the single-core Bass program (same program on all 8 cores)."""
    f32 = mybir.dt.float32
    bf16 = mybir.dt.bfloat16
    DT = D // P            # 8 d-tiles (contraction over D)
    KT = S // P            # 16 k-token tiles per batch
    TTI = T // P           # 32 token tiles total
    NOT = D // P           # 8 output tiles for Wo projection
    QC = S // q_chunk      # q chunks per batch
    QH = q_chunk // 512    # 512-wide halves per q chunk

    nc = bacc.Bacc(None, target_bir_lowering=False, debug=False)

    # ---- I/O ----
    xt_d = nc.declare_dram_parameter("xt", [D, T], bf16, isOutput=False)
    wq_d = nc.declare_dram_parameter("wq", [D, DSH], bf16, isOutput=False)
    bq_d = nc.declare_dram_parameter("bq", [DSH, 1], f32, isOutput=False)
    wk_d = nc.declare_dram_parameter("wk", [D, DSH], bf16, isOutput=False)
    bk_d = nc.declare_dram_parameter("bk", [DSH, 1], f32, isOutput=False)
    wv_d = nc.declare_dram_parameter("wv", [D, VW], bf16, isOutput=False)
    bv_d = nc.declare_dram_parameter("bv", [1, VW], bf16, isOutput=False)
    wo_d = nc.declare_dram_parameter("wo", [DSH, D], bf16, isOutput=False)
    attn_d = nc.declare_dram_parameter("attn", [B, HPC, S, S], bf16, isOutput=True)
    out_d = nc.declare_dram_parameter("out", [D, T], bf16, isOutput=True)

    AF = mybir.ActivationFunctionType
    OP = mybir.AluOpType

    with tile.TileContext(nc) as tc:
        with (
            tc.tile_pool(name="const", bufs=1) as cpool,
            tc.tile_pool(name="big", bufs=1) as bpool,
        ):
            # persistent SBUF tensors
            wq_sb = cpool.tile([P, DT, DSH], bf16, tag="wq")
            wk_sb = cpool.tile([P, DT, DSH], bf16, tag="wk")
            wv_sb = cpool.tile([P, DT, VW], bf16, tag="wv")
            wo_sb = cpool.tile([P, D], bf16, tag="wo")
            bq_sb = cpool.tile([P, 1], f32, tag="bq")
            bk_sb = cpool.tile([P, 1], f32, tag="bk")
            bvb_sb = cpool.tile([P, VW], bf16, tag="bvb")

            qt_sb = bpool.tile([P, T], bf16, tag="qt")
            kt_sb = bpool.tile([P, T], bf16, tag="kt")
            v_sb = bpool.tile([P, TTI, VW], bf16, tag="v")
            ctxt_sb = bpool.tile([P, T], bf16, tag="ctxt")

            nc.sync.dma_start(wq_sb[:], wq_d.rearrange("(dt p) m -> p dt m", p=P))
            nc.sync.dma_start(wk_sb[:], wk_d.rearrange("(dt p) m -> p dt m", p=P))
            nc.sync.dma_start(wv_sb[:], wv_d.rearrange("(dt p) m -> p dt m", p=P))
            nc.sync.dma_start(wo_sb[:], wo_d[:])
            nc.sync.dma_start(bq_sb[:], bq_d[:])
            nc.sync.dma_start(bk_sb[:], bk_d[:])
            nc.sync.dma_start(bvb_sb[:], bv_d[:].to_broadcast((P, VW)))

            # ACT warmup: force the exp table-set load onto a wait-free inst
            warm_sb = cpool.tile([1, 8], f32, tag="warm")
            nc.vector.memset(warm_sb[:], 0.0)
            nc.scalar.activation(warm_sb[:], warm_sb[:], AF.Exp)

            # ---------------- Phase A: projections ----------------
            with (
                tc.tile_pool(name="xt", bufs=1) as xpool,
                tc.tile_pool(name="psA", bufs=2, space="PSUM") as psA,
            ):
                xt_r = xt_d.rearrange("(dt p) t -> p dt t", p=P)
                xt_tiles = []
                for ch in range(T // 512):
                    xt_c = xpool.tile([P, DT, 512], bf16, tag=f"xt{ch}", name=f"xt{ch}")
                    nc.sync.dma_start(xt_c[:], xt_r[:, :, ch * 512 : (ch + 1) * 512])
                    xt_tiles.append(xt_c)

                # QT, KT: [dsh, T], accumulate over 8 d-tiles per 512-chunk
                for w_sb, b_sb, dst in ((wq_sb, bq_sb, qt_sb), (wk_sb, bk_sb, kt_sb)):
                    for ch in range(T // 512):
                        ps = psA.tile([P, 512], f32, tag="psqk")
                        sl = slice(ch * 512, (ch + 1) * 512)
                        for dt in range(DT):
                            nc.tensor.matmul(
                                ps[:],
                                lhsT=w_sb[:, dt, :],
                                rhs=xt_tiles[ch][:, dt, :],
                                start=(dt == 0),
                                stop=(dt == DT - 1),
                            )
                        nc.vector.tensor_tensor(
                            dst[:, sl],
                            ps[:],
                            b_sb[:, 0:1].to_broadcast((P, 512)),
                            OP.add,
                        )

                # V: token-major [tok, VW] with ones cols; bias via broadcast add
                for tt in range(TTI):
                    psv = psA.tile([P, VW], f32, tag="psv")
                    for dt in range(DT):
                        nc.tensor.matmul(
                            psv[:],
                            lhsT=xt_tiles[tt // 4][:, dt, (tt % 4) * P : (tt % 4 + 1) * P],
                            rhs=wv_sb[:, dt, :],
                            start=(dt == 0),
                            stop=(dt == DT - 1),
                        )
                    nc.vector.tensor_tensor(v_sb[:, tt, :], psv[:], bvb_sb[:], OP.add)

            # ---------------- Phase B: attention ----------------
            with (
                tc.tile_pool(name="exp", bufs=2) as epool,
                tc.tile_pool(name="ost", bufs=4) as opool,
                tc.tile_pool(name="bc", bufs=1) as bcpool,
                tc.tile_pool(name="dbc", bufs=2, space="DRAM") as dpool,
                tc.tile_pool(name="psS", bufs=1, space="PSUM") as psS,
                tc.tile_pool(name="psC", bufs=1, space="PSUM") as psC,
            ):
                for b in range(B):
                    for qc in range(QC):
                        q0 = b * S + qc * q_chunk  # token offset of q chunk
                        exps = [
                            epool.tile([P, KT, q_chunk], bf16, tag=f"exp{hl}", name=f"exp{hl}")
                            for hl in range(HPC)
                        ]
                        # scores + exp, k-tile by k-tile, both heads row-packed
                        for kt in range(KT):
                            ksl = slice(b * S + kt * P, b * S + (kt + 1) * P)
                            pss = [
                                psS.tile([P, q_chunk], f32, tag=f"S{hl}", name=f"S{hl}")
                                for hl in range(HPC)
                            ]
                            for hl in range(HPC):
                                hp = slice(hl * DK, (hl + 1) * DK)
                                for qh in range(QH):
                                    qsl = slice(q0 + qh * 512, q0 + (qh + 1) * 512)
                                    nc.tensor.matmul(
                                        pss[hl][:, qh * 512 : (qh + 1) * 512],
                                        lhsT=kt_sb[hp, ksl],
                                        rhs=qt_sb[hp, qsl],
                                        start=True,
                                        stop=True,
                                        tile_position=(hl * DK, 0),
                                    )
                            for hl in range(HPC):
                                nc.scalar.activation(
                                    exps[hl][:, kt, :], pss[hl][:], AF.Exp
                                )
                        # attn @ [V | 1]: accumulate ctx + row sums over k-tiles
                        cacc = [
                            [
                                psC.tile(
                                    [P, 512],
                                    f32,
                                    tag=f"C{hl * QH + qh}",
                                    name=f"C{hl}{qh}",
                                )[: DK + 1, :]
                                for qh in range(QH)
                            ]
                            for hl in range(HPC)
                        ]
                        for kt in range(KT):
                            vt = b * KT + kt
                            for hl in range(HPC):
                                vsl = slice(hl * (DK + 1), (hl + 1) * (DK + 1))
                                for qh in range(QH):
                                    nc.tensor.matmul(
                                        cacc[hl][qh][:],
                                        lhsT=v_sb[:, vt, vsl],
                                        rhs=exps[hl][:, kt, qh * 512 : (qh + 1) * 512],
                                        start=(kt == 0),
                                        stop=(kt == KT - 1),
                                    )
                        # normalize + writeout per head
                        for hl in range(HPC):
                            sums = bcpool.tile([1, q_chunk], f32, tag=f"sum{hl}")
                            bcf = bcpool.tile([P, q_chunk], f32, tag=f"bcf{hl}")
                            bc16 = bcpool.tile([P, q_chunk], bf16, tag=f"bc16{hl}")
                            for qh in range(QH):
                                nc.vector.tensor_copy(
                                    out=sums[0:1, qh * 512 : (qh + 1) * 512],
                                    in_=cacc[hl][qh][DK : DK + 1, :],
                                )
                            sums_dr = dpool.tile(
                                [1, q_chunk], f32, tag=f"sd{hl}", name=f"sd{hl}"
                            )
                            nc.sync.dma_start(sums_dr[:], sums[0:1, :])
                            nc.sync.dma_start(
                                bcf[:], sums_dr[:].to_broadcast((P, q_chunk))
                            )
                            nc.vector.reciprocal_approx_fast(out=bcf[:], in_=bcf[:])
                            nc.vector.tensor_copy(out=bc16[:], in_=bcf[:])
                            # attn = exp * (1/sum) in place, then DMA out;
                            # quartered so the DMA drains while later
                            # quarters still normalize
                            attn_r = attn_d[
                                b, hl, :, qc * q_chunk : (qc + 1) * q_chunk
                            ].rearrange("(kt p) q -> p kt q", p=P)
                            NQ = 4
                            for nq in range(NQ):
                                kq = slice(nq * KT // NQ, (nq + 1) * KT // NQ)
                                nc.vector.tensor_tensor(
                                    exps[hl][:, kq, :],
                                    exps[hl][:, kq, :],
                                    bc16[:, None, :].to_broadcast(
                                        (P, KT // NQ, q_chunk)
                                    ),
                                    OP.mult,
                                )
                                nc.sync.dma_start(
                                    attn_r[:, kq, :], exps[hl][:, kq, :]
                                )
                            # ctxT = unnorm_ctx * (1/sum)
                            for qh in range(QH):
                                csl = slice(q0 + qh * 512, q0 + (qh + 1) * 512)
                                nc.vector.tensor_tensor(
                                    ctxt_sb[hl * DK : (hl + 1) * DK, csl],
                                    cacc[hl][qh][0:DK, :],
                                    bc16[0:DK, qh * 512 : (qh + 1) * 512],
                                    OP.mult,
                                )
                        # output projection for this instance's tokens,
                        # interleaved to keep PE warm; partial out goes to
                        # HBM straight from PSUM (bias folded on host)
                        for ot in range(NOT):
                            osl = slice(ot * P, (ot + 1) * P)
                            for qh in range(QH):
                                csl = slice(q0 + qh * 512, q0 + (qh + 1) * 512)
                                pso = psC.tile(
                                    [P, 512],
                                    f32,
                                    tag=f"C{(ot * QH + qh) % 4}",
                                    name=f"pso{ot}{qh}",
                                )
                                nc.tensor.matmul(
                                    pso[:],
                                    lhsT=wo_sb[:, osl],
                                    rhs=ctxt_sb[:, csl],
                                    start=True,
                                    stop=True,
                                )
                                ou = opool.tile(
                                    [P, 512], bf16, tag="ou", name="ou"
                                )
                                if (ot + qh) % 2 == 0:
                                    nc.scalar.copy(ou[:], pso[:])
                                else:
                                    nc.vector.tensor_copy(out=ou[:], in_=pso[:])
                                nc.sync.dma_start(out_d[osl, csl], ou[:])

    nc.compile()
    return nc


_NC_CACHE = {}


def _get_program():
    if "nc" not in _NC_CACHE:
        _NC_CACHE["nc"] = build_program()
    return _NC_CACHE["nc"]


def _prep_inputs(x, Wq, bq, Wk, bk, Wv, bv, Wo, bo):
    """Host-side sharding + layout prep. Returns in_maps (one dict per core)."""
    scale = 1.0 / np.sqrt(np.float32(DK))  # folded into Wq/bq
    xt = np.ascontiguousarray(x.reshape(T, D).T).astype(BF16)
    in_maps = []
    for c in range(N_CORES):
        sl = slice(DSH * c, DSH * (c + 1))
        wq_c = np.ascontiguousarray((Wq[sl] * scale).T).astype(BF16)
        bq_c = (bq[sl] * scale).astype(np.float32)[:, None]
        wk_c = np.ascontiguousarray(Wk[sl].T).astype(BF16)
        bk_c = bk[sl].astype(np.float32)[:, None]
        wv_c = np.zeros((D, VW), np.float32)
        bv_c = np.zeros((1, VW), np.float32)
        for hl in range(HPC):
            rows = slice(DSH * c + hl * DK, DSH * c + (hl + 1) * DK)
            cols = slice(hl * (DK + 1), hl * (DK + 1) + DK)
            wv_c[:, cols] = Wv[rows].T
            bv_c[0, cols] = bv[rows]
            bv_c[0, hl * (DK + 1) + DK] = 1.0
        wo_c = np.ascontiguousarray(Wo[:, sl].T).astype(BF16)
        in_maps.append(
            {
                "xt": xt,
                "wq": wq_c,
                "bq": bq_c,
                "wk": wk_c,
                "bk": bk_c,
                "wv": wv_c.astype(BF16),
                "bv": bv_c.astype(BF16),
                "wo": wo_c,
            }
        )
    return in_maps


def kernel(x, Wq, bq, Wk, bk, Wv, bv, Wo, bo):
    global LAST_RESULTS
    from concourse.bass_utils import run_bass_kernel_spmd

    x = np.asarray(x, np.float32)
    in_maps = _prep_inputs(
        x,
        np.asarray(Wq, np.float32),
        np.asarray(bq, np.float32),
        np.asarray(Wk, np.float32),
        np.asarray(bk, np.float32),
        np.asarray(Wv, np.float32),
        np.asarray(bv, np.float32),
        np.asarray(Wo, np.float32),
        np.asarray(bo, np.float32),
    )
    nc = _get_program()
    res = run_bass_kernel_spmd(nc, in_maps, list(range(N_CORES)), trace=TRACE)
    LAST_RESULTS = res

    # ---- gather / unshard ----
    out_t = np.zeros((D, T), np.float32)
    out_t += np.asarray(bo, np.float32).reshape(1, D).T  # bias once, post-reduce
    attn = np.empty((B, H, S, S), np.float32)
    for c in range(N_CORES):
        r = res.results[c]
        out_t += np.asarray(r["out"]).astype(np.float32)
        a = np.asarray(r["attn"])  # [B, HPC, S(k), S(q)] bf16
        for hl in range(HPC):
            attn[:, HPC * c + hl] = np.swapaxes(
                a[:, hl].astype(np.float32), 1, 2
            )
    output = np.ascontiguousarray(out_t.T).reshape(B, S, D)
    return output, attn


# revision 16
# speedup vs baseline: 1.1000x; 1.0926x over previous
"""Multi-head self-attention Trainium2 kernel (8 NeuronCores, head-parallel).

B=2, S=2048, D=1024, H=16, DK=64. Each core owns 2 heads (a 128-wide slice of
the QKV output dims / Wo input dims) and both batch elements. Inputs are
pre-sliced + pre-transposed on the host; partial Wo outputs are summed on the
host. No cross-core communication.

Device-side layout (all bf16 on PE, fp32 PSUM accumulation):
  xT   [D, T]        activations feature-major (T = B*S tokens)
  QT,KT[128, T]      per-core head dims on partitions
  V    [tok, dk]     token-major, augmented with a ones column per head so the
                     attn @ V matmul also yields softmax denominators
  scores/exp/attn    [k, q] tiles (k on partitions) -> feeds attn @ V directly
  ctxT [128, T]      context feature-major -> feeds Wo projection
  outT [D, T]        per-core partial of the final projection
"""

import os
import sys

import numpy as np

for _p in ("/opt/trn_rl_repo",):
    if _p not in sys.path and os.path.isdir(_p):
        sys.path.insert(0, _p)

import ml_dtypes  # noqa: E402

import concourse.bass as bass  # noqa: E402
import concourse.mybir as mybir  # noqa: E402
import concourse.tile as tile  # noqa: E402
from concourse import bacc  # noqa: E402

BF16 = ml_dtypes.bfloat16
P = 128

# Problem config (hardcoded per spec)
B, S, D, H = 2, 2048, 1024, 16
DK = D // H            # 64
N_CORES = 8
HPC = H // N_CORES     # heads per core = 2
DSH = HPC * DK         # head dims per core = 128
T = B * S              # 4096 tokens
VW = DSH + HPC         # V width incl. ones cols = 130

TRACE = False
LAST_RESULTS = None


def build_program(q_chunk=1024):
    """Build the single-core Bass program (same program on all 8 cores)."""
    f32 = mybir.dt.float32
    bf16 = mybir.dt.bfloat16
    DT = D // P            # 8 d-tiles (contraction over D)
    KT = S // P            # 16 k-token tiles per batch
    TTI = T // P           # 32 token tiles total
    NOT = D // P           # 8 output tiles for Wo projection
    QC = S // q_chunk      # q chunks per batch
    QH = q_chunk // 512    # 512-wide halves per q chunk

    nc = bacc.Bacc(None, target_bir_lowering=False, debug=False)

    # ---- I/O ----
    xt_d = nc.declare_dram_parameter("xt", [D, T], bf16, isOutput=False)
    wq_d = nc.declare_dram_parameter("wq", [D, DSH], bf16, isOutput=False)
    bq_d = nc.declare_dram_parameter("bq", [DSH, 1], f32, isOutput=False)
    wk_d = nc.declare_dram_parameter("wk", [D, DSH], bf16, isOutput=False)
    bk_d = nc.declare_dram_parameter("bk", [DSH, 1], f32, isOutput=False)
    wv_d = nc.declare_dram_parameter("wv", [D, VW], bf16, isOutput=False)
    bv_d = nc.declare_dram_parameter("bv", [1, VW], bf16, isOutput=False)
    wo_d = nc.declare_dram_parameter("wo", [DSH, D], bf16, isOutput=False)
    attn_d = nc.declare_dram_parameter("attn", [B, HPC, S, S], bf16, isOutput=True)
    out_d = nc.declare_dram_parameter("out", [D, T], bf16, isOutput=True)

    AF = mybir.ActivationFunctionType
    OP = mybir.AluOpType

    with tile.TileContext(nc) as tc:
        with (
            tc.tile_pool(name="const", bufs=1) as cpool,
            tc.tile_pool(name="big", bufs=1) as bpool,
        ):
            # persistent SBUF tensors
            wq_sb = cpool.tile([P, DT, DSH], bf16, tag="wq")
            wk_sb = cpool.tile([P, DT, DSH], bf16, tag="wk")
            wv_sb = cpool.tile([P, DT, VW], bf16, tag="wv")
            wo_sb = cpool.tile([P, D], bf16, tag="wo")
            bq_sb = cpool.tile([P, 1], f32, tag="bq")
            bk_sb = cpool.tile([P, 1], f32, tag="bk")
            bvb_sb = cpool.tile([P, VW], bf16, tag="bvb")

            qt_sb = bpool.tile([P, T], bf16, tag="qt")
            kt_sb = bpool.tile([P, T], bf16, tag="kt")
            v_sb = bpool.tile([P, TTI, VW], bf16, tag="v")
            ctxt_sb = bpool.tile([P, T], bf16, tag="ctxt")

            nc.sync.dma_start(wq_sb[:], wq_d.rearrange("(dt p) m -> p dt m", p=P))
            nc.sync.dma_start(wk_sb[:], wk_d.rearrange("(dt p) m -> p dt m", p=P))
            nc.sync.dma_start(wv_sb[:], wv_d.rearrange("(dt p) m -> p dt m", p=P))
            nc.sync.dma_start(wo_sb[:], wo_d[:])
            nc.sync.dma_start(bq_sb[:], bq_d[:])
            nc.sync.dma_start(bk_sb[:], bk_d[:])
            nc.sync.dma_start(bvb_sb[:], bv_d[:].to_broadcast((P, VW)))

            # ACT warmup: force the exp table-set load onto a wait-free inst
            warm_sb = cpool.tile([1, 8], f32, tag="warm")
            nc.vector.memset(warm_sb[:], 0.0)
            nc.scalar.activation(warm_sb[:], warm_sb[:], AF.Exp)

            # ---------------- Phase A: projections ----------------
            with (
                tc.tile_pool(name="xt", bufs=1) as xpool,
                tc.tile_pool(name="psA", bufs=2, space="PSUM") as psA,
            ):
                xt_r = xt_d.rearrange("(dt p) t -> p dt t", p=P)
                xt_tiles = []
                for ch in range(T // 512):
                    xt_c = xpool.tile([P, DT, 512], bf16, tag=f"xt{ch}", name=f"xt{ch}")
                    nc.sync.dma_start(xt_c[:], xt_r[:, :, ch * 512 : (ch + 1) * 512])
                    xt_tiles.append(xt_c)

                # QT, KT: [dsh, T], accumulate over 8 d-tiles per 512-chunk
                for w_sb, b_sb, dst in ((wq_sb, bq_sb, qt_sb), (wk_sb, bk_sb, kt_sb)):
                    for ch in range(T // 512):
                        ps = psA.tile([P, 512], f32, tag="psqk")
                        sl = slice(ch * 512, (ch + 1) * 512)
                        for dt in range(DT):
                            nc.tensor.matmul(
                                ps[:],
                                lhsT=w_sb[:, dt, :],
                                rhs=xt_tiles[ch][:, dt, :],
                                start=(dt == 0),
                                stop=(dt == DT - 1),
                            )
                        nc.vector.tensor_tensor(
                            dst[:, sl],
                            ps[:],
                            b_sb[:, 0:1].to_broadcast((P, 512)),
                            OP.add,
                        )

                # V: token-major [tok, VW] with ones cols; bias via broadcast add
                for tt in range(TTI):
                    psv = psA.tile([P, VW], f32, tag="psv")
                    for dt in range(DT):
                        nc.tensor.matmul(
                            psv[:],
                            lhsT=xt_tiles[tt // 4][:, dt, (tt % 4) * P : (tt % 4 + 1) * P],
                            rhs=wv_sb[:, dt, :],
                            start=(dt == 0),
                            stop=(dt == DT - 1),
                        )
                    nc.vector.tensor_tensor(v_sb[:, tt, :], psv[:], bvb_sb[:], OP.add)

            # ---------------- Phase B: attention ----------------
            with (
                tc.tile_pool(name="exp", bufs=2) as epool,
                tc.tile_pool(name="bc", bufs=1) as bcpool,
                tc.tile_pool(name="dbc", bufs=2, space="DRAM") as dpool,
                tc.tile_pool(name="psS", bufs=1, space="PSUM") as psS,
                tc.tile_pool(name="psC", bufs=1, space="PSUM") as psC,
            ):
                for b in range(B):
                    for qc in range(QC):
                        q0 = b * S + qc * q_chunk  # token offset of q chunk
                        exps = [
                            epool.tile(
                                [P, KT, q_chunk],
                                bf16,
                                tag=f"exp{hl}",
                                name=f"exp{hl}",
                            )
                            for hl in range(HPC)
                        ]
                        # ctx accumulators [dk+1, q_chunk] per head (row DK =
                        # softmax denominators via the ones column of V)
                        cacc = [
                            psC.tile(
                                [P, q_chunk], f32, tag=f"C{hl}", name=f"C{hl}"
                            )[: DK + 1, :]
                            for hl in range(HPC)
                        ]
                        # k-tile pipeline: scores (PE) -> exp (ACT) -> ctx
                        # accumulate (PE). Interleaving ctx per k-tile keeps
                        # PE duty dense (HAM stays warm) and lets ACT stream
                        # exps back-to-back across instances.
                        for kt in range(KT):
                            ksl = slice(b * S + kt * P, b * S + (kt + 1) * P)
                            vt = b * KT + kt
                            pss = [
                                psS.tile(
                                    [P, q_chunk], f32, tag=f"S{hl}", name=f"S{hl}"
                                )
                                for hl in range(HPC)
                            ]
                            for hl in range(HPC):
                                hp = slice(hl * DK, (hl + 1) * DK)
                                for qh in range(QH):
                                    qsl = slice(q0 + qh * 512, q0 + (qh + 1) * 512)
                                    nc.tensor.matmul(
                                        pss[hl][:, qh * 512 : (qh + 1) * 512],
                                        lhsT=kt_sb[hp, ksl],
                                        rhs=qt_sb[hp, qsl],
                                        start=True,
                                        stop=True,
                                        tile_position=(hl * DK, 0),
                                    )
                            for hl in range(HPC):
                                nc.scalar.activation(
                                    exps[hl][:, kt, :], pss[hl][:], AF.Exp
                                )
                            for hl in range(HPC):
                                vsl = slice(hl * (DK + 1), (hl + 1) * (DK + 1))
                                for qh in range(QH):
                                    nc.tensor.matmul(
                                        cacc[hl][:, qh * 512 : (qh + 1) * 512],
                                        lhsT=v_sb[:, vt, vsl],
                                        rhs=exps[hl][:, kt, qh * 512 : (qh + 1) * 512],
                                        start=(kt == 0),
                                        stop=(kt == KT - 1),
                                    )
                        # normalize + writeout per head (runs on DVE/DMA,
                        # overlapped with the next instance's score/exp/ctx)
                        for hl in range(HPC):
                            sums = bcpool.tile([1, q_chunk], f32, tag=f"sum{hl}")
                            bcf = bcpool.tile([P, q_chunk], f32, tag=f"bcf{hl}")
                            bc16 = bcpool.tile([P, q_chunk], bf16, tag=f"bc16{hl}")
                            nc.vector.tensor_copy(
                                out=sums[0:1, :], in_=cacc[hl][DK : DK + 1, :]
                            )
                            sums_dr = dpool.tile(
                                [1, q_chunk], f32, tag=f"sd{hl}", name=f"sd{hl}"
                            )
                            nc.sync.dma_start(sums_dr[:], sums[0:1, :])
                            nc.sync.dma_start(
                                bcf[:], sums_dr[:].to_broadcast((P, q_chunk))
                            )
                            nc.vector.reciprocal_approx_fast(out=bcf[:], in_=bcf[:])
                            nc.vector.tensor_copy(out=bc16[:], in_=bcf[:])
                            # attn = exp * (1/sum) in place, then DMA out;
                            # quartered so the DMA drains early
                            attn_r = attn_d[
                                b, hl, :, qc * q_chunk : (qc + 1) * q_chunk
                            ].rearrange("(kt p) q -> p kt q", p=P)
                            NQ = 4
                            for nq in range(NQ):
                                kq = slice(nq * KT // NQ, (nq + 1) * KT // NQ)
                                nc.vector.tensor_tensor(
                                    exps[hl][:, kq, :],
                                    exps[hl][:, kq, :],
                                    bc16[:, None, :].to_broadcast(
                                        (P, KT // NQ, q_chunk)
                                    ),
                                    OP.mult,
                                )
                                nc.sync.dma_start(
                                    attn_r[:, kq, :], exps[hl][:, kq, :]
                                )
                            # ctxT = unnorm_ctx * (1/sum)
                            nc.vector.tensor_tensor(
                                ctxt_sb[
                                    hl * DK : (hl + 1) * DK, q0 : q0 + q_chunk
                                ],
                                cacc[hl][0:DK, :],
                                bc16[0:DK, :],
                                OP.mult,
                            )

            # ---------------- Phase C: output projection ----------------
            with (
                tc.tile_pool(name="psO", bufs=4, space="PSUM") as psO,
                tc.tile_pool(name="ost", bufs=6) as opool,
            ):
                for ot in range(NOT):
                    osl = slice(ot * P, (ot + 1) * P)
                    for ch in range(T // 512):
                        sl = slice(ch * 512, (ch + 1) * 512)
                        pso = psO.tile([P, 512], f32, tag="pso")
                        nc.tensor.matmul(
                            pso[:],
                            lhsT=wo_sb[:, osl],
                            rhs=ctxt_sb[:, sl],
                            start=True,
                            stop=True,
                        )
                        ou = opool.tile([P, 512], bf16, tag="ou", name="ou")
                        if (ot + ch) % 2 == 0:
                            nc.scalar.copy(ou[:], pso[:])
                        else:
                            nc.vector.tensor_copy(out=ou[:], in_=pso[:])
                        nc.sync.dma_start(out_d[osl, sl], ou[:])

    nc.compile()
    return nc


_NC_CACHE = {}


def _get_program():
    if "nc" not in _NC_CACHE:
        _NC_CACHE["nc"] = build_program()
    return _NC_CACHE["nc"]


def _prep_inputs(x, Wq, bq, Wk, bk, Wv, bv, Wo, bo):
    """Host-side sharding + layout prep. Returns in_maps (one dict per core)."""
    scale = 1.0 / np.sqrt(np.float32(DK))  # folded into Wq/bq
    xt = np.ascontiguousarray(x.reshape(T, D).T).astype(BF16)
    in_maps = []
    for c in range(N_CORES):
        sl = slice(DSH * c, DSH * (c + 1))
        wq_c = np.ascontiguousarray((Wq[sl] * scale).T).astype(BF16)
        bq_c = (bq[sl] * scale).astype(np.float32)[:, None]
        wk_c = np.ascontiguousarray(Wk[sl].T).astype(BF16)
        bk_c = bk[sl].astype(np.float32)[:, None]
        wv_c = np.zeros((D, VW), np.float32)
        bv_c = np.zeros((1, VW), np.float32)
        for hl in range(HPC):
            rows = slice(DSH * c + hl * DK, DSH * c + (hl + 1) * DK)
            cols = slice(hl * (DK + 1), hl * (DK + 1) + DK)
            wv_c[:, cols] = Wv[rows].T
            bv_c[0, cols] = bv[rows]
            bv_c[0, hl * (DK + 1) + DK] = 1.0
        wo_c = np.ascontiguousarray(Wo[:, sl].T).astype(BF16)
        in_maps.append(
            {
                "xt": xt,
                "wq": wq_c,
                "bq": bq_c,
                "wk": wk_c,
                "bk": bk_c,
                "wv": wv_c.astype(BF16),
                "bv": bv_c.astype(BF16),
                "wo": wo_c,
            }
        )
    return in_maps


def kernel(x, Wq, bq, Wk, bk, Wv, bv, Wo, bo):
    global LAST_RESULTS
    from concourse.bass_utils import run_bass_kernel_spmd

    x = np.asarray(x, np.float32)
    in_maps = _prep_inputs(
        x,
        np.asarray(Wq, np.float32),
        np.asarray(bq, np.float32),
        np.asarray(Wk, np.float32),
        np.asarray(bk, np.float32),
        np.asarray(Wv, np.float32),
        np.asarray(bv, np.float32),
        np.asarray(Wo, np.float32),
        np.asarray(bo, np.float32),
    )
    nc = _get_program()
    res = run_bass_kernel_spmd(nc, in_maps, list(range(N_CORES)), trace=TRACE)
    LAST_RESULTS = res

    # ---- gather / unshard ----
    out_t = np.zeros((D, T), np.float32)
    out_t += np.asarray(bo, np.float32).reshape(1, D).T  # bias once, post-reduce
    attn = np.empty((B, H, S, S), np.float32)
    for c in range(N_CORES):
        r = res.results[c]
        out_t += np.asarray(r["out"]).astype(np.float32)
        a = np.asarray(r["attn"])  # [B, HPC, S(k), S(q)] bf16
        for hl in range(HPC):
            attn[:, HPC * c + hl] = np.swapaxes(
                a[:, hl].astype(np.float32), 1, 2
            )
    output = np.ascontiguousarray(out_t.T).reshape(B, S, D)
    return output, attn
